# revision 9
# baseline (speedup 1.0000x reference)
"""Differential attention (DiffAttn) kernel for 8 TRN2 NeuronCores.

Problem: B=4, T=4096, C=1024, one differential head (2x64 qk dims, 128 v dims),
causal, weights = softmax(q1k1/8) - lam * softmax(q2k2/8), out = weights @ v.

Sharding: pure data-parallel, zero collectives. 8 cores = 4 batches x 2
query-halves. The query rows are zigzag-interleaved at 256-row granularity
(core half h owns rows [512k + 256h, 512k + 256h + 256) for k=0..7) so both
halves have identical causal tile structure (SPMD: one graph for all cores)
and identical FLOPs.

Per-core pipeline (bf16 compute, fp32 accumulation):
  - host pre-swizzles x^T into [128p, 8c, 8sb, 512] so each 512-key block is
    ONE 512KB DMA; kv projection starts as soon as block 0 lands. Host also
    gathers the core's own query columns (xq, [128p, 8c, 4tb, 512]),
    pre-scales Wq by 1/8, computes lam, and builds the causal mask constants.
  - projections on PE: kT[128f, T], qT[128f, 2048] (feature-major = scores
    operand layout) and v[s, 128] (via vT + DMA-transpose).
  - scores in [t, s] layout, both heads sequentially through ONE 4-bank
    PSUM tile [128, 4, 512] -> exp groups of up to 4 chunks (48 ACTIVATEs
    total) with accum_out row sums (no max-subtraction needed: scores are
    ~N(0,1) so exp never overflows; softmax is shift-invariant).
  - causal mask: host-built additive -30000 mask on the diagonal 512-chunk.
  - combine in ONE fused DVE op: p_neg = p2 * (lam*sum1/sum2) - p1
    (per-partition scalar), then DMA-transpose (xbar) the combined strip,
    PV matmul (interleaved into later subtiles' score stream so PE never
    head-of-line-stalls on the exp->combine->transpose chain), and a final
    fused scale by -1/sum1 on eviction.
"""
import math
import os
import sys
import types
from contextlib import ExitStack

import ml_dtypes
import numpy as np


def _install_ntff_hook():
    """Make `antenv.axon_hooks` importable (the agent image ships a stub
    antenv without it), wiring the NTFF profile hook straight to the axon
    .so so run_bass_kernel_spmd(trace=True) can report HW exec time."""
    try:
        import antenv.axon_hooks  # noqa: F401
        return
    except Exception:
        pass
    try:
        import antenv
    except Exception:
        return
    mod = types.ModuleType("antenv.axon_hooks")
    mod._hook = None

    def set_axon_ntff_profile_hook(h):
        mod._hook = h

    def get_axon_ntff_profile_hook():
        if mod._hook is None:
            try:
                from trn_agent_boot.trn_boot import _ntff_profile_via_ctypes
                mod._hook = _ntff_profile_via_ctypes("/opt/axon/libaxon_pjrt.so")
            except Exception:
                mod._hook = None
        return mod._hook

    mod.set_axon_ntff_profile_hook = set_axon_ntff_profile_hook
    mod.get_axon_ntff_profile_hook = get_axon_ntff_profile_hook
    sys.modules["antenv.axon_hooks"] = mod
    antenv.axon_hooks = mod


_install_ntff_hook()

import concourse.bacc as bacc
import concourse.bass as bass
import concourse.bass_utils as _bass_utils
import concourse.tile as tile
from concourse import mybir
from concourse.bass_utils import run_bass_kernel_spmd

# zero-egress container: don't try to copy NEFF/NTFF artifacts to a bucket
_bass_utils.upload_artifacts = lambda tmpdir: f"local://{tmpdir}"

BF16 = mybir.dt.bfloat16
F32 = mybir.dt.float32
NPBF16 = ml_dtypes.bfloat16
ts = bass.ts

B, T, C = 4, 4096, 1024
HS, H2 = 64, 128
NSUB = 16          # 128-row query subtiles per core
ROWS = NSUB * 128  # 2048 query rows per core
MASK_NEG = -30000.0

LAST_EXEC_NS = None
_NC_CACHE = {}


def _t0(j, half):
    """Global first query row of subtile j on core-half `half`."""
    return 512 * (j // 2) + 128 * (j % 2) + 256 * half


def _build(lam: float):
    nc = bacc.Bacc()
    xT_e = nc.declare_dram_parameter("xT", [128, 8, 8, 512], BF16, isOutput=False)
    xq_e = nc.declare_dram_parameter("xq", [128, 8, 4, 512], BF16, isOutput=False)
    wq_e = nc.declare_dram_parameter("wq", [C, H2], BF16, isOutput=False)
    wk_e = nc.declare_dram_parameter("wk", [C, H2], BF16, isOutput=False)
    wv_e = nc.declare_dram_parameter("wv", [C, H2], BF16, isOutput=False)
    cm_e = nc.declare_dram_parameter("cmask", [2, 128, 512], BF16, isOutput=False)
    out_e = nc.declare_dram_parameter("out", [NSUB, 128, H2], BF16, isOutput=True)

    Exp = mybir.ActivationFunctionType.Exp
    mult = mybir.AluOpType.mult
    sub = mybir.AluOpType.subtract
    add = mybir.AluOpType.add

    with ExitStack() as ctx:
        tc = ctx.enter_context(tile.TileContext(nc))
        const = ctx.enter_context(tc.tile_pool(name="const", bufs=1))
        persist = ctx.enter_context(tc.tile_pool(name="persist", bufs=1))
        vt_pool = ctx.enter_context(tc.tile_pool(name="vt", bufs=2))
        p_pool = ctx.enter_context(tc.tile_pool(name="p", bufs=2))
        pt_pool = ctx.enter_context(tc.tile_pool(name="pt", bufs=3))
        small = ctx.enter_context(tc.tile_pool(name="small", bufs=4))
        osb_pool = ctx.enter_context(tc.tile_pool(name="osb", bufs=2))
        proj_ps = ctx.enter_context(tc.tile_pool(name="proj_ps", bufs=1, space="PSUM"))
        sc_ps = ctx.enter_context(tc.tile_pool(name="sc_ps", bufs=1, space="PSUM"))
        pv_ps = ctx.enter_context(tc.tile_pool(name="pv_ps", bufs=1, space="PSUM"))

        # --- constants + resident x^T / xq ---
        # weights + mask first on sync (small, needed by first matmuls)
        wq_sb = const.tile([128, 8, 128], BF16)
        wk_sb = const.tile([128, 8, 128], BF16)
        wv_sb = const.tile([128, 8, 128], BF16)
        for c in range(8):
            nc.sync.dma_start(wk_sb[:, c, :], wk_e[ts(c, 128), :])
            nc.sync.dma_start(wv_sb[:, c, :], wv_e[ts(c, 128), :])
            nc.sync.dma_start(wq_sb[:, c, :], wq_e[ts(c, 128), :])
        cm_sb = const.tile([128, 2, 512], BF16)
        for m in range(2):
            nc.sync.dma_start(cm_sb[:, m, :], cm_e[m, :, :])
        # x^T, one 512KB DMA per 512-key block, alternating queues so the
        # first blocks land fast; xq interleaved on the other queue.
        xt_sb = const.tile([128, 8, 8, 512], BF16)   # [p, c, sb, col]
        xq_sb = const.tile([128, 8, 4, 512], BF16)   # [p, c, tb, col]
        nc.gpsimd.dma_start(xt_sb[:, :, 0, :], xT_e[:, :, 0, :])
        nc.scalar.dma_start(xq_sb[:, :, 0, :], xq_e[:, :, 0, :])
        nc.gpsimd.dma_start(xt_sb[:, :, 1, :], xT_e[:, :, 1, :])
        nc.scalar.dma_start(xt_sb[:, :, 2, :], xT_e[:, :, 2, :])
        nc.gpsimd.dma_start(xt_sb[:, :, 3, :], xT_e[:, :, 3, :])
        nc.scalar.dma_start(xq_sb[:, :, 1, :], xq_e[:, :, 1, :])
        nc.gpsimd.dma_start(xt_sb[:, :, 4, :], xT_e[:, :, 4, :])
        nc.scalar.dma_start(xt_sb[:, :, 5, :], xT_e[:, :, 5, :])
        nc.gpsimd.dma_start(xt_sb[:, :, 6, :], xT_e[:, :, 6, :])
        nc.scalar.dma_start(xq_sb[:, :, 2, :], xq_e[:, :, 2, :])
        nc.gpsimd.dma_start(xt_sb[:, :, 7, :], xT_e[:, :, 7, :])
        nc.scalar.dma_start(xq_sb[:, :, 3, :], xq_e[:, :, 3, :])

        # --- persistent projection outputs ---
        qT = persist.tile([128, ROWS], BF16)     # [q-feature, own t]
        kT = persist.tile([128, T], BF16)        # [k-feature, s]
        v_sb = persist.tile([128, 32, 128], BF16)  # [s%128, s//128, v-feature]

        # PE's per-engine instruction stream is static and in-order, so ALL
        # deferrable PE work (projection matmuls + PV matmuls of subtile j-2)
        # goes into one FIFO of closures, drained inside the exp windows of
        # the score loop: PE never idles waiting for ACT, and never sees a
        # >3.4us gap (which would drop the HAM clock gate back to 1.2 GHz).
        filler = []
        popped = [0]
        appended = [0]

        def push(fn):
            filler.append(fn)
            appended[0] += 1

        def fill(n):
            while n > 0 and filler:
                filler.pop(0)()
                popped[0] += 1
                n -= 1

        def drain_to(mark):
            while popped[0] < mark and filler:
                filler.pop(0)()
                popped[0] += 1

        def proj_block(w_sb, rhs_of_c, done):
            ps_box = []

            def mk(c):
                def go():
                    if c == 0:
                        ps_box.append(proj_ps.tile([128, 512], F32,
                                                   name="pp", tag="pp"))
                    nc.tensor.matmul(ps_box[0][:], w_sb[:, c, :], rhs_of_c(c),
                                     start=(c == 0), stop=(c == 7))
                    if c == 7:
                        done(ps_box[0])
                return go

            for c in range(8):
                push(mk(c))

        def q_done(tb):
            def done(ps):
                nc.vector.tensor_copy(qT[:, ts(tb, 512)], ps[:])
            return done

        def k_done(sb):
            def done(ps):
                nc.vector.tensor_copy(kT[:, ts(sb, 512)], ps[:])
            return done

        def v_done(sb):
            def done(ps):
                vt = vt_pool.tile([128, 512], BF16)
                nc.vector.tensor_copy(vt[:], ps[:])
                nc.sync.dma_start_transpose(v_sb[:, 4 * sb:4 * sb + 4, :], vt[:])
            return done

        # supply schedule: k(sb) [q(tb)] v(sb); marks record the FIFO position
        # whose drain guarantees kT(sb) / qT(tb) writes are emitted (Tile
        # derives dependencies from emission order, so consumers must be
        # emitted after producers).
        k_mark = {}
        q_mark = {}
        for sb in range(8):
            proj_block(wk_sb, lambda c, s=sb: xt_sb[:, c, s, :], k_done(sb))
            k_mark[sb] = appended[0]
            if sb < 4:
                proj_block(wq_sb, lambda c, t=sb: xq_sb[:, c, t, :], q_done(sb))
                q_mark[sb] = appended[0]
            proj_block(wv_sb, lambda c, s=sb: xt_sb[:, c, s, :], v_done(sb))

        def attention_scores(j):
            nch = j // 2 + 1          # 512-wide key chunks covered
            ngr = (nch + 3) // 4      # 4-chunk exp groups per head
            p1 = p_pool.tile([128, nch, 512], BF16, tag="p1")
            p2 = p_pool.tile([128, nch, 512], BF16, tag="p2")
            sp1 = small.tile([128, 2], F32, tag="sp1")
            sp2 = small.tile([128, 2], F32, tag="sp2")
            for gi in range(ngr):
                used = min(4, nch - 4 * gi)
                for h, (p, sp) in ((0, (p1, sp1)), (1, (p2, sp2))):
                    sc = sc_ps.tile([128, 4, 512], F32, tag="sc")
                    for qd in range(used):
                        ch = 4 * gi + qd
                        nc.tensor.matmul(
                            sc[:, qd, :],
                            qT[64 * h:64 * h + 64, ts(j, 128)],
                            kT[64 * h:64 * h + 64, ts(ch, 512)],
                            start=True, stop=True)
                    if 4 * gi + used == nch:  # strip's diagonal chunk
                        nc.vector.tensor_add(sc[:, used - 1, :],
                                             sc[:, used - 1, :],
                                             cm_sb[:, j % 2, :])
                    nc.scalar.activation(p[:, 4 * gi:4 * gi + used, :],
                                         sc[:, 0:used, :], Exp,
                                         accum_out=sp[:, gi:gi + 1])
                    # cover the (used*512+352)/1.2 ns exp window with ~200ns
                    # filler slots so PE stays busy while ACT runs
                    fill(max(2, (used * 512 + 352) // 220))
            if ngr == 1:
                sum1, sum2 = sp1[:, 0:1], sp2[:, 0:1]
            else:
                s1t = small.tile([128, 1], F32, tag="s1t")
                s2t = small.tile([128, 1], F32, tag="s2t")
                nc.vector.tensor_add(s1t[:], sp1[:, 0:1], sp1[:, 1:2])
                nc.vector.tensor_add(s2t[:], sp2[:, 0:1], sp2[:, 1:2])
                sum1, sum2 = s1t[:], s2t[:]
            r2 = small.tile([128, 1], F32, tag="r2")
            r1 = small.tile([128, 1], F32, tag="r1")
            gsc = small.tile([128, 1], F32, tag="gsc")
            nc.vector.reciprocal(r2[:], sum2)
            nc.vector.reciprocal(r1[:], sum1)
            # gsc = lam * sum1 / sum2
            nc.vector.scalar_tensor_tensor(gsc[:], sum1, float(lam), r2[:],
                                           op0=mult, op1=mult)
            # p_neg = p2 * gsc - p1   (one fused DVE pass over the strip)
            pn = p_pool.tile([128, nch, 512], BF16, tag="pn")
            nc.vector.scalar_tensor_tensor(pn[:], p2[:, 0:nch, :], gsc[:],
                                           p1[:, 0:nch, :], op0=mult, op1=sub)
            # transpose the whole combined strip in one xbar DMA
            pt = pt_pool.tile([128, 4 * nch, 128], BF16)
            nc.sync.dma_start_transpose(pt[:], pn[:])
            return pt, r1, nch

        def queue_pv(j, pt, r1, nch):
            pv_box = []

            def mk_mm(cc):
                def go():
                    if cc == 0:
                        pv_box.append(pv_ps.tile([128, 128], F32,
                                                 name="pv", tag="pv"))
                    nc.tensor.matmul(pv_box[0][:], pt[:, cc, :], v_sb[:, cc, :],
                                     start=(cc == 0), stop=(cc == 4 * nch - 1))
                return go

            def finish():
                osb = osb_pool.tile([128, 128], BF16)
                # out = pv * r1 * (-1)  (fused negate undoes the p_neg sign)
                nc.vector.tensor_scalar(osb[:], pv_box[0][:], r1[:], -1.0,
                                        op0=mult, op1=mult)
                nc.gpsimd.dma_start(out_e[j, :, :], osb[:])

            for cc in range(4 * nch):
                push(mk_mm(cc))
            push(finish)

        lagged = []

        def run_subtile(j):
            # kT/qT producer closures for this subtile must be emitted first
            drain_to(max(k_mark[j // 2], q_mark[j // 4]))
            # bound the PV queue: at most two subtiles pending.
            if len(lagged) >= 2:
                queue_pv(*lagged.pop(0))
            res = attention_scores(j)
            lagged.append((j, *res))
            fill(4)

        for j in range(NSUB):
            run_subtile(j)
        while lagged:
            queue_pv(*lagged.pop(0))
            fill(len(filler))
        fill(len(filler))

    nc.compile()
    return nc


def _lambda_init(depth):
    return 0.8 - 0.6 * math.exp(-0.3 * (depth + 1))


def kernel(x, Wq, Wk, Wv, lambda_q1, lambda_q2, lambda_k1, lambda_k2):
    global LAST_EXEC_NS
    x = np.asarray(x, dtype=np.float32)
    Wq = np.asarray(Wq, dtype=np.float32)
    Wk = np.asarray(Wk, dtype=np.float32)
    Wv = np.asarray(Wv, dtype=np.float32)
    lq1 = np.asarray(lambda_q1, dtype=np.float64)
    lq2 = np.asarray(lambda_q2, dtype=np.float64)
    lk1 = np.asarray(lambda_k1, dtype=np.float64)
    lk2 = np.asarray(lambda_k2, dtype=np.float64)

    lam = float(np.exp(np.dot(lq1, lk1)) - np.exp(np.dot(lq2, lk2))
                + _lambda_init(0))

    key = round(lam, 9)
    if key not in _NC_CACHE:
        _NC_CACHE[key] = _build(lam)
    nc = _NC_CACHE[key]

    wq_h = np.ascontiguousarray((Wq * 0.125).astype(NPBF16))
    wk_h = np.ascontiguousarray(Wk.astype(NPBF16))
    wv_h = np.ascontiguousarray(Wv.astype(NPBF16))

    # x^T per batch in bf16, then swizzle to [128p, 8c, 8sb, 512]
    xT = [x[b].T.astype(NPBF16) for b in range(B)]
    xt_h = [np.ascontiguousarray(
        t.reshape(8, 128, 8, 512).transpose(1, 0, 2, 3)) for t in xT]

    i_idx = np.arange(128)[:, None]
    j_idx = np.arange(512)[None, :]
    in_maps = []
    for core in range(8):
        b, half = core // 2, core % 2
        qcols = np.concatenate(
            [np.arange(_t0(j, half), _t0(j, half) + 128) for j in range(NSUB)])
        xq = np.ascontiguousarray(
            xT[b][:, qcols].reshape(8, 128, 4, 512).transpose(1, 0, 2, 3))
        cm = np.empty((2, 128, 512), dtype=NPBF16)
        for m in range(2):
            r = 128 * m + 256 * half
            cm[m] = np.where(i_idx + r >= j_idx, 0.0, MASK_NEG).astype(NPBF16)
        in_maps.append({"xT": xt_h[b], "xq": xq, "wq": wq_h, "wk": wk_h,
                        "wv": wv_h, "cmask": cm})

    try:
        res = run_bass_kernel_spmd(nc, in_maps, list(range(8)))
    except Exception:
        if os.environ.get("BASS_TRACE"):
            # profiling path failed; rerun untraced
            os.environ["BASS_NEVER_TRACE"] = "1"
            res = run_bass_kernel_spmd(nc, in_maps, list(range(8)))
        else:
            raise
    LAST_EXEC_NS = res.exec_time_ns

    out = np.empty((B, T, H2), dtype=np.float32)
    for core in range(8):
        b, half = core // 2, core % 2
        o = np.asarray(res.results[core]["out"]).astype(np.float32)
        for j in range(NSUB):
            t0 = _t0(j, half)
            out[b, t0:t0 + 128, :] = o[j]
    return out


# revision 11
# speedup vs baseline: 1.3293x; 1.3293x over previous
"""Differential attention (DiffAttn) kernel for 8 TRN2 NeuronCores.

Problem: B=4, T=4096, C=1024, one differential head (2x64 qk dims, 128 v dims),
causal, weights = softmax(q1k1/8) - lam * softmax(q2k2/8), out = weights @ v.

Sharding: pure data-parallel, zero collectives. 8 cores = 4 batches x 2
query-halves. The query rows are zigzag-interleaved at 256-row granularity
(core half h owns rows [512k + 256h, 512k + 256h + 256) for k=0..7) so both
halves have identical causal tile structure (SPMD: one graph for all cores)
and identical FLOPs.

Per-core pipeline (bf16 compute, fp32 accumulation):
  - host pre-swizzles x^T into [128p, 8c, 8sb, 512] so each 512-key block is
    ONE 512KB DMA; kv projection starts as soon as block 0 lands. Host also
    gathers the core's own query columns (xq, [128p, 8c, 4tb, 512]),
    pre-scales Wq by 1/8, computes lam, and builds the causal mask constants.
  - projections on PE: kT[128f, T], qT[128f, 2048] (feature-major = scores
    operand layout) and v[s, 128] (via vT + DMA-transpose).
  - scores in [t, s] layout, both heads sequentially through ONE 4-bank
    PSUM tile [128, 4, 512] -> exp groups of up to 4 chunks (48 ACTIVATEs
    total) with accum_out row sums (no max-subtraction needed: scores are
    ~N(0,1) so exp never overflows; softmax is shift-invariant).
  - causal mask: host-built additive -30000 mask on the diagonal 512-chunk.
  - combine in ONE fused DVE op: p_neg = p2 * (lam*sum1/sum2) - p1
    (per-partition scalar), then DMA-transpose (xbar) the combined strip,
    PV matmul (interleaved into later subtiles' score stream so PE never
    head-of-line-stalls on the exp->combine->transpose chain), and a final
    fused scale by -1/sum1 on eviction.
"""
import math
import os
import sys
import types
from contextlib import ExitStack

import ml_dtypes
import numpy as np


def _install_ntff_hook():
    """Make `antenv.axon_hooks` importable (the agent image ships a stub
    antenv without it), wiring the NTFF profile hook straight to the axon
    .so so run_bass_kernel_spmd(trace=True) can report HW exec time."""
    try:
        import antenv.axon_hooks  # noqa: F401
        return
    except Exception:
        pass
    try:
        import antenv
    except Exception:
        return
    mod = types.ModuleType("antenv.axon_hooks")
    mod._hook = None

    def set_axon_ntff_profile_hook(h):
        mod._hook = h

    def get_axon_ntff_profile_hook():
        if mod._hook is None:
            try:
                from trn_agent_boot.trn_boot import _ntff_profile_via_ctypes
                mod._hook = _ntff_profile_via_ctypes("/opt/axon/libaxon_pjrt.so")
            except Exception:
                mod._hook = None
        return mod._hook

    mod.set_axon_ntff_profile_hook = set_axon_ntff_profile_hook
    mod.get_axon_ntff_profile_hook = get_axon_ntff_profile_hook
    sys.modules["antenv.axon_hooks"] = mod
    antenv.axon_hooks = mod


_install_ntff_hook()

import concourse.bacc as bacc
import concourse.bass as bass
import concourse.bass_utils as _bass_utils
import concourse.tile as tile
from concourse import mybir
from concourse.bass_utils import run_bass_kernel_spmd

# zero-egress container: don't try to copy NEFF/NTFF artifacts to a bucket
_bass_utils.upload_artifacts = lambda tmpdir: f"local://{tmpdir}"

BF16 = mybir.dt.bfloat16
F32 = mybir.dt.float32
NPBF16 = ml_dtypes.bfloat16
ts = bass.ts

B, T, C = 4, 4096, 1024
HS, H2 = 64, 128
NSUB = 16          # 128-row query subtiles per core
ROWS = NSUB * 128  # 2048 query rows per core
MASK_NEG = -30000.0

LAST_EXEC_NS = None
_NC_CACHE = {}


def _t0(j, half):
    """Global first query row of subtile j on core-half `half`."""
    return 512 * (j // 2) + 128 * (j % 2) + 256 * half


def _build(lam: float):
    nc = bacc.Bacc()
    xT_e = nc.declare_dram_parameter("xT", [128, 8, 8, 512], BF16, isOutput=False)
    xq_e = nc.declare_dram_parameter("xq", [128, 8, 4, 512], BF16, isOutput=False)
    wq_e = nc.declare_dram_parameter("wq", [C, H2], BF16, isOutput=False)
    wk_e = nc.declare_dram_parameter("wk", [C, H2], BF16, isOutput=False)
    wv_e = nc.declare_dram_parameter("wv", [C, H2], BF16, isOutput=False)
    cm_e = nc.declare_dram_parameter("cmask", [2, 128, 512], BF16, isOutput=False)
    out_e = nc.declare_dram_parameter("out", [NSUB, 128, H2], BF16, isOutput=True)

    Exp = mybir.ActivationFunctionType.Exp
    mult = mybir.AluOpType.mult
    sub = mybir.AluOpType.subtract
    add = mybir.AluOpType.add

    with ExitStack() as ctx:
        tc = ctx.enter_context(tile.TileContext(nc))
        const = ctx.enter_context(tc.tile_pool(name="const", bufs=1))
        persist = ctx.enter_context(tc.tile_pool(name="persist", bufs=1))
        vt_pool = ctx.enter_context(tc.tile_pool(name="vt", bufs=2))
        p_pool = ctx.enter_context(tc.tile_pool(name="p", bufs=2))
        pt_pool = ctx.enter_context(tc.tile_pool(name="pt", bufs=3))
        small = ctx.enter_context(tc.tile_pool(name="small", bufs=4))
        osb_pool = ctx.enter_context(tc.tile_pool(name="osb", bufs=2))
        proj_ps = ctx.enter_context(tc.tile_pool(name="proj_ps", bufs=1, space="PSUM"))
        sc_ps = ctx.enter_context(tc.tile_pool(name="sc_ps", bufs=1, space="PSUM"))
        pv_ps = ctx.enter_context(tc.tile_pool(name="pv_ps", bufs=1, space="PSUM"))

        # --- constants + resident x^T / xq ---
        # weights + mask first on sync (small, needed by first matmuls)
        wq_sb = const.tile([128, 8, 128], BF16)
        wk_sb = const.tile([128, 8, 128], BF16)
        wv_sb = const.tile([128, 8, 128], BF16)
        for c in range(8):
            nc.sync.dma_start(wk_sb[:, c, :], wk_e[ts(c, 128), :])
            nc.sync.dma_start(wv_sb[:, c, :], wv_e[ts(c, 128), :])
            nc.sync.dma_start(wq_sb[:, c, :], wq_e[ts(c, 128), :])
        cm_sb = const.tile([128, 2, 512], BF16)
        for m in range(2):
            nc.sync.dma_start(cm_sb[:, m, :], cm_e[m, :, :])
        # x^T, one 512KB DMA per 512-key block, alternating queues so the
        # first blocks land fast; xq interleaved on the other queue.
        xt_sb = const.tile([128, 8, 8, 512], BF16)   # [p, c, sb, col]
        xq_sb = const.tile([128, 8, 4, 512], BF16)   # [p, c, tb, col]
        nc.gpsimd.dma_start(xt_sb[:, :, 0, :], xT_e[:, :, 0, :])
        nc.scalar.dma_start(xq_sb[:, :, 0, :], xq_e[:, :, 0, :])
        nc.gpsimd.dma_start(xt_sb[:, :, 1, :], xT_e[:, :, 1, :])
        nc.scalar.dma_start(xt_sb[:, :, 2, :], xT_e[:, :, 2, :])
        nc.gpsimd.dma_start(xt_sb[:, :, 3, :], xT_e[:, :, 3, :])
        nc.scalar.dma_start(xq_sb[:, :, 1, :], xq_e[:, :, 1, :])
        nc.gpsimd.dma_start(xt_sb[:, :, 4, :], xT_e[:, :, 4, :])
        nc.scalar.dma_start(xt_sb[:, :, 5, :], xT_e[:, :, 5, :])
        nc.gpsimd.dma_start(xt_sb[:, :, 6, :], xT_e[:, :, 6, :])
        nc.scalar.dma_start(xq_sb[:, :, 2, :], xq_e[:, :, 2, :])
        nc.gpsimd.dma_start(xt_sb[:, :, 7, :], xT_e[:, :, 7, :])
        nc.scalar.dma_start(xq_sb[:, :, 3, :], xq_e[:, :, 3, :])

        # --- persistent projection outputs ---
        qT = persist.tile([128, ROWS], BF16)     # [q-feature, own t]
        kT = persist.tile([128, T], BF16)        # [k-feature, s]
        v_sb = persist.tile([128, 32, 128], BF16)  # [s%128, s//128, v-feature]

        # PE's per-engine instruction stream is static and in-order, so ALL
        # deferrable PE work (projection matmuls + PV matmuls of subtile j-2)
        # goes into one FIFO of closures, drained inside the exp windows of
        # the score loop: PE never idles waiting for ACT, and never sees a
        # >3.4us gap (which would drop the HAM clock gate back to 1.2 GHz).
        filler = []
        popped = [0]
        appended = [0]

        def push(fn):
            filler.append(fn)
            appended[0] += 1

        def fill(n):
            while n > 0 and filler:
                filler.pop(0)()
                popped[0] += 1
                n -= 1

        def drain_to(mark):
            while popped[0] < mark and filler:
                filler.pop(0)()
                popped[0] += 1

        def proj_block(w_sb, rhs_of_c, done):
            ps_box = []

            def mk(c):
                def go():
                    if c == 0:
                        ps_box.append(proj_ps.tile([128, 512], F32,
                                                   name="pp", tag="pp"))
                    nc.tensor.matmul(ps_box[0][:], w_sb[:, c, :], rhs_of_c(c),
                                     start=(c == 0), stop=(c == 7))
                    if c == 7:
                        done(ps_box[0])
                return go

            for c in range(8):
                push(mk(c))

        def q_done(tb):
            def done(ps):
                nc.vector.tensor_copy(qT[:, ts(tb, 512)], ps[:])
            return done

        def k_done(sb):
            def done(ps):
                nc.vector.tensor_copy(kT[:, ts(sb, 512)], ps[:])
            return done

        def v_done(sb):
            def done(ps):
                vt = vt_pool.tile([128, 512], BF16)
                nc.vector.tensor_copy(vt[:], ps[:])
                nc.sync.dma_start_transpose(v_sb[:, 4 * sb:4 * sb + 4, :], vt[:])
            return done

        # supply schedule: k(sb) [q(tb)] v(sb); marks record the FIFO position
        # whose drain guarantees kT(sb) / qT(tb) writes are emitted (Tile
        # derives dependencies from emission order, so consumers must be
        # emitted after producers).
        k_mark = {}
        q_mark = {}
        for sb in range(8):
            proj_block(wk_sb, lambda c, s=sb: xt_sb[:, c, s, :], k_done(sb))
            k_mark[sb] = appended[0]
            if sb < 4:
                proj_block(wq_sb, lambda c, t=sb: xq_sb[:, c, t, :], q_done(sb))
                q_mark[sb] = appended[0]
            proj_block(wv_sb, lambda c, s=sb: xt_sb[:, c, s, :], v_done(sb))

        def attention_scores(j):
            nch = j // 2 + 1          # 512-wide key chunks covered
            ngr = (nch + 2) // 3      # 3-chunk exp groups per head
            p1 = p_pool.tile([128, nch, 512], BF16, tag="p1")
            p2 = p_pool.tile([128, nch, 512], BF16, tag="p2")
            sp1 = small.tile([128, 3], F32, tag="sp1")
            sp2 = small.tile([128, 3], F32, tag="sp2")
            for gi in range(ngr):
                used = min(3, nch - 3 * gi)
                for h, (p, sp) in ((0, (p1, sp1)), (1, (p2, sp2))):
                    sc = sc_ps.tile([128, 3, 512], F32, tag=f"sc{h}")
                    for qd in range(used):
                        ch = 3 * gi + qd
                        nc.tensor.matmul(
                            sc[:, qd, :],
                            qT[64 * h:64 * h + 64, ts(j, 128)],
                            kT[64 * h:64 * h + 64, ts(ch, 512)],
                            start=True, stop=True)
                    if 3 * gi + used == nch:  # strip's diagonal chunk
                        nc.vector.tensor_add(sc[:, used - 1, :],
                                             sc[:, used - 1, :],
                                             cm_sb[:, j % 2, :])
                    nc.scalar.activation(p[:, 3 * gi:3 * gi + used, :],
                                         sc[:, 0:used, :], Exp,
                                         accum_out=sp[:, gi:gi + 1])
                    # cover the (used*512+352)/1.2 ns exp window with ~200ns
                    # filler slots so PE stays busy while ACT runs
                    fill(max(2, (used * 512 + 352) // 220))
            if ngr == 1:
                sum1, sum2 = sp1[:, 0:1], sp2[:, 0:1]
            else:
                s1t = small.tile([128, 1], F32, tag="s1t")
                s2t = small.tile([128, 1], F32, tag="s2t")
                nc.vector.tensor_reduce(s1t[:], sp1[:, 0:ngr],
                                        axis=mybir.AxisListType.X, op=add)
                nc.vector.tensor_reduce(s2t[:], sp2[:, 0:ngr],
                                        axis=mybir.AxisListType.X, op=add)
                sum1, sum2 = s1t[:], s2t[:]
            r2 = small.tile([128, 1], F32, tag="r2")
            r1 = small.tile([128, 1], F32, tag="r1")
            gsc = small.tile([128, 1], F32, tag="gsc")
            nc.vector.reciprocal(r2[:], sum2)
            nc.vector.reciprocal(r1[:], sum1)
            # gsc = lam * sum1 / sum2
            nc.vector.scalar_tensor_tensor(gsc[:], sum1, float(lam), r2[:],
                                           op0=mult, op1=mult)
            # p_neg = p2 * gsc - p1   (one fused DVE pass over the strip)
            pn = p_pool.tile([128, nch, 512], BF16, tag="pn")
            nc.vector.scalar_tensor_tensor(pn[:], p2[:, 0:nch, :], gsc[:],
                                           p1[:, 0:nch, :], op0=mult, op1=sub)
            return pn, r1, nch

        def queue_pv(j, pt, r1, nch):
            pv_box = []

            def mk_mm(cc):
                def go():
                    if cc == 0:
                        pv_box.append(pv_ps.tile([128, 128], F32,
                                                 name="pv", tag="pv"))
                    nc.tensor.matmul(pv_box[0][:], pt[:, cc, :], v_sb[:, cc, :],
                                     start=(cc == 0), stop=(cc == 4 * nch - 1))
                return go

            def finish():
                osb = osb_pool.tile([128, 128], BF16)
                # out = pv * r1 * (-1)  (fused negate undoes the p_neg sign)
                nc.vector.tensor_scalar(osb[:], pv_box[0][:], r1[:], -1.0,
                                        op0=mult, op1=mult)
                nc.gpsimd.dma_start(out_e[j, :, :], osb[:])

            for cc in range(4 * nch):
                push(mk_mm(cc))
            push(finish)

        pending_t = []   # (j, pn, r1, nch) awaiting transpose emission
        pending_pv = []  # (j, pt, r1, nch) awaiting PV queueing

        def emit_transpose(ent):
            # lag-1 emission: pn is complete by now, so the xbar transpose
            # never head-of-line-blocks the sync HWDGE ring (which also
            # carries the v transposes PV depends on)
            j, pn, r1, nch = ent
            pt = pt_pool.tile([128, 4 * nch, 128], BF16, name="pt")
            nc.sync.dma_start_transpose(pt[:], pn[:])
            pending_pv.append((j, pt, r1, nch))

        def run_subtile(j):
            # kT/qT producer closures for this subtile must be emitted first
            drain_to(max(k_mark[j // 2], q_mark[j // 4]))
            if pending_t:
                emit_transpose(pending_t.pop(0))
            if len(pending_pv) >= 2:
                queue_pv(*pending_pv.pop(0))
            res = attention_scores(j)
            pending_t.append((j, *res))
            fill(4)

        for j in range(NSUB):
            run_subtile(j)
        while pending_t:
            emit_transpose(pending_t.pop(0))
        while pending_pv:
            queue_pv(*pending_pv.pop(0))
            fill(len(filler))
        fill(len(filler))

    nc.compile()
    return nc


def _lambda_init(depth):
    return 0.8 - 0.6 * math.exp(-0.3 * (depth + 1))


def kernel(x, Wq, Wk, Wv, lambda_q1, lambda_q2, lambda_k1, lambda_k2):
    global LAST_EXEC_NS
    x = np.asarray(x, dtype=np.float32)
    Wq = np.asarray(Wq, dtype=np.float32)
    Wk = np.asarray(Wk, dtype=np.float32)
    Wv = np.asarray(Wv, dtype=np.float32)
    lq1 = np.asarray(lambda_q1, dtype=np.float64)
    lq2 = np.asarray(lambda_q2, dtype=np.float64)
    lk1 = np.asarray(lambda_k1, dtype=np.float64)
    lk2 = np.asarray(lambda_k2, dtype=np.float64)

    lam = float(np.exp(np.dot(lq1, lk1)) - np.exp(np.dot(lq2, lk2))
                + _lambda_init(0))

    key = round(lam, 9)
    if key not in _NC_CACHE:
        _NC_CACHE[key] = _build(lam)
    nc = _NC_CACHE[key]

    wq_h = np.ascontiguousarray((Wq * 0.125).astype(NPBF16))
    wk_h = np.ascontiguousarray(Wk.astype(NPBF16))
    wv_h = np.ascontiguousarray(Wv.astype(NPBF16))

    # x^T per batch in bf16, then swizzle to [128p, 8c, 8sb, 512]
    xT = [x[b].T.astype(NPBF16) for b in range(B)]
    xt_h = [np.ascontiguousarray(
        t.reshape(8, 128, 8, 512).transpose(1, 0, 2, 3)) for t in xT]

    i_idx = np.arange(128)[:, None]
    j_idx = np.arange(512)[None, :]
    in_maps = []
    for core in range(8):
        b, half = core // 2, core % 2
        qcols = np.concatenate(
            [np.arange(_t0(j, half), _t0(j, half) + 128) for j in range(NSUB)])
        xq = np.ascontiguousarray(
            xT[b][:, qcols].reshape(8, 128, 4, 512).transpose(1, 0, 2, 3))
        cm = np.empty((2, 128, 512), dtype=NPBF16)
        for m in range(2):
            r = 128 * m + 256 * half
            cm[m] = np.where(i_idx + r >= j_idx, 0.0, MASK_NEG).astype(NPBF16)
        in_maps.append({"xT": xt_h[b], "xq": xq, "wq": wq_h, "wk": wk_h,
                        "wv": wv_h, "cmask": cm})

    try:
        res = run_bass_kernel_spmd(nc, in_maps, list(range(8)))
    except Exception:
        if os.environ.get("BASS_TRACE"):
            # profiling path failed; rerun untraced
            os.environ["BASS_NEVER_TRACE"] = "1"
            res = run_bass_kernel_spmd(nc, in_maps, list(range(8)))
        else:
            raise
    LAST_EXEC_NS = res.exec_time_ns

    out = np.empty((B, T, H2), dtype=np.float32)
    for core in range(8):
        b, half = core // 2, core % 2
        o = np.asarray(res.results[core]["out"]).astype(np.float32)
        for j in range(NSUB):
            t0 = _t0(j, half)
            out[b, t0:t0 + 128, :] = o[j]
    return out


# revision 15
# speedup vs baseline: 1.3983x; 1.0519x over previous
"""Differential attention (DiffAttn) kernel for 8 TRN2 NeuronCores.

Problem: B=4, T=4096, C=1024, one differential head (2x64 qk dims, 128 v dims),
causal, weights = softmax(q1k1/8) - lam * softmax(q2k2/8), out = weights @ v.

Sharding: pure data-parallel, zero collectives. 8 cores = 4 batches x 2
query-halves. The query rows are zigzag-interleaved at 256-row granularity
(core half h owns rows [512k + 256h, 512k + 256h + 256) for k=0..7) so both
halves have identical causal tile structure (SPMD: one graph for all cores)
and identical FLOPs.

Per-core pipeline (bf16 compute, fp32 accumulation):
  - host pre-swizzles x^T into [128p, 8c, 8sb, 512] so each 512-key block is
    ONE 512KB DMA; kv projection starts as soon as block 0 lands. Host also
    gathers the core's own query columns (xq, [128p, 8c, 4tb, 512]),
    pre-scales Wq by 1/8, computes lam, and builds the causal mask constants.
  - projections on PE: kT[128f, T], qT[128f, 2048] (feature-major = scores
    operand layout) and v[s, 128] (via vT + DMA-transpose).
  - scores in [t, s] layout, both heads sequentially through ONE 4-bank
    PSUM tile [128, 4, 512] -> exp groups of up to 4 chunks (48 ACTIVATEs
    total) with accum_out row sums (no max-subtraction needed: scores are
    ~N(0,1) so exp never overflows; softmax is shift-invariant).
  - causal mask: host-built additive -30000 mask on the diagonal 512-chunk.
  - combine in ONE fused DVE op: p_neg = p2 * (lam*sum1/sum2) - p1
    (per-partition scalar), then DMA-transpose (xbar) the combined strip,
    PV matmul (interleaved into later subtiles' score stream so PE never
    head-of-line-stalls on the exp->combine->transpose chain), and a final
    fused scale by -1/sum1 on eviction.
"""
import math
import os
import sys
import types
from contextlib import ExitStack

import ml_dtypes
import numpy as np


def _install_ntff_hook():
    """Make `antenv.axon_hooks` importable (the agent image ships a stub
    antenv without it), wiring the NTFF profile hook straight to the axon
    .so so run_bass_kernel_spmd(trace=True) can report HW exec time."""
    try:
        import antenv.axon_hooks  # noqa: F401
        return
    except Exception:
        pass
    try:
        import antenv
    except Exception:
        return
    mod = types.ModuleType("antenv.axon_hooks")
    mod._hook = None

    def set_axon_ntff_profile_hook(h):
        mod._hook = h

    def get_axon_ntff_profile_hook():
        if mod._hook is None:
            try:
                from trn_agent_boot.trn_boot import _ntff_profile_via_ctypes
                mod._hook = _ntff_profile_via_ctypes("/opt/axon/libaxon_pjrt.so")
            except Exception:
                mod._hook = None
        return mod._hook

    mod.set_axon_ntff_profile_hook = set_axon_ntff_profile_hook
    mod.get_axon_ntff_profile_hook = get_axon_ntff_profile_hook
    sys.modules["antenv.axon_hooks"] = mod
    antenv.axon_hooks = mod


_install_ntff_hook()

import concourse.bacc as bacc
import concourse.bass as bass
import concourse.bass_utils as _bass_utils
import concourse.tile as tile
from concourse import mybir
from concourse.bass_utils import run_bass_kernel_spmd

# zero-egress container: don't try to copy NEFF/NTFF artifacts to a bucket
_bass_utils.upload_artifacts = lambda tmpdir: f"local://{tmpdir}"

BF16 = mybir.dt.bfloat16
F32 = mybir.dt.float32
NPBF16 = ml_dtypes.bfloat16
ts = bass.ts

B, T, C = 4, 4096, 1024
HS, H2 = 64, 128
NSUB = 16          # 128-row query subtiles per core
ROWS = NSUB * 128  # 2048 query rows per core
MASK_NEG = -30000.0

LAST_EXEC_NS = None
_NC_CACHE = {}


def _t0(j, half):
    """Global first query row of subtile j on core-half `half`."""
    return 512 * (j // 2) + 128 * (j % 2) + 256 * half


def _build(lam: float):
    nc = bacc.Bacc()
    xT_e = nc.declare_dram_parameter("xT", [128, 8, 8, 512], BF16, isOutput=False)
    xq_e = nc.declare_dram_parameter("xq", [128, 8, 4, 512], BF16, isOutput=False)
    wq_e = nc.declare_dram_parameter("wq", [128, 8, 128], BF16, isOutput=False)
    wk_e = nc.declare_dram_parameter("wk", [128, 8, 128], BF16, isOutput=False)
    wv_e = nc.declare_dram_parameter("wv", [128, 8, 128], BF16, isOutput=False)
    cm_e = nc.declare_dram_parameter("cmask", [128, 2, 512], BF16, isOutput=False)
    out_e = nc.declare_dram_parameter("out", [NSUB, 128, H2], BF16, isOutput=True)

    Exp = mybir.ActivationFunctionType.Exp
    mult = mybir.AluOpType.mult
    sub = mybir.AluOpType.subtract
    add = mybir.AluOpType.add

    with ExitStack() as ctx:
        tc = ctx.enter_context(tile.TileContext(nc))
        const = ctx.enter_context(tc.tile_pool(name="const", bufs=1))
        persist = ctx.enter_context(tc.tile_pool(name="persist", bufs=1))
        vt_pool = ctx.enter_context(tc.tile_pool(name="vt", bufs=2))
        p_pool = ctx.enter_context(tc.tile_pool(name="p", bufs=2))
        pt_pool = ctx.enter_context(tc.tile_pool(name="pt", bufs=3))
        small = ctx.enter_context(tc.tile_pool(name="small", bufs=4))
        osb_pool = ctx.enter_context(tc.tile_pool(name="osb", bufs=2))
        proj_ps = ctx.enter_context(tc.tile_pool(name="proj_ps", bufs=1, space="PSUM"))
        sc_ps = ctx.enter_context(tc.tile_pool(name="sc_ps", bufs=1, space="PSUM"))
        pv_ps = ctx.enter_context(tc.tile_pool(name="pv_ps", bufs=1, space="PSUM"))

        # --- constants + resident x^T / xq ---
        # weights + mask host-swizzled so each is ONE DMA (many small DMAs
        # serialize ~2us apiece through the 8 HWDGE completion lanes)
        wq_sb = const.tile([128, 8, 128], BF16)
        wk_sb = const.tile([128, 8, 128], BF16)
        wv_sb = const.tile([128, 8, 128], BF16)
        cm_sb = const.tile([128, 2, 512], BF16)
        nc.sync.dma_start(wk_sb[:], wk_e[:])
        nc.sync.dma_start(wq_sb[:], wq_e[:])
        nc.sync.dma_start(wv_sb[:], wv_e[:])
        nc.sync.dma_start(cm_sb[:], cm_e[:])
        # x^T, one 512KB DMA per 512-key block, alternating queues so the
        # first blocks land fast; xq interleaved on the other queue.
        xt_sb = const.tile([128, 8, 8, 512], BF16)   # [p, c, sb, col]
        xq_sb = const.tile([128, 8, 4, 512], BF16)   # [p, c, tb, col]
        nc.gpsimd.dma_start(xt_sb[:, :, 0, :], xT_e[:, :, 0, :])
        nc.scalar.dma_start(xq_sb[:, :, 0, :], xq_e[:, :, 0, :])
        nc.gpsimd.dma_start(xt_sb[:, :, 2, :], xT_e[:, :, 2, :])
        nc.scalar.dma_start(xt_sb[:, :, 1, :], xT_e[:, :, 1, :])
        nc.gpsimd.dma_start(xt_sb[:, :, 4, :], xT_e[:, :, 4, :])
        nc.scalar.dma_start(xq_sb[:, :, 1, :], xq_e[:, :, 1, :])
        nc.gpsimd.dma_start(xt_sb[:, :, 6, :], xT_e[:, :, 6, :])
        nc.scalar.dma_start(xt_sb[:, :, 3, :], xT_e[:, :, 3, :])
        nc.scalar.dma_start(xq_sb[:, :, 2, :], xq_e[:, :, 2, :])
        nc.scalar.dma_start(xt_sb[:, :, 5, :], xT_e[:, :, 5, :])
        nc.scalar.dma_start(xq_sb[:, :, 3, :], xq_e[:, :, 3, :])
        nc.scalar.dma_start(xt_sb[:, :, 7, :], xT_e[:, :, 7, :])

        # --- persistent projection outputs ---
        qT = persist.tile([128, ROWS], BF16)     # [q-feature, own t]
        kT = persist.tile([128, T], BF16)        # [k-feature, s]
        v_sb = persist.tile([128, 32, 128], BF16)  # [s%128, s//128, v-feature]

        # PE's per-engine instruction stream is static and in-order, so ALL
        # deferrable PE work (projection matmuls + PV matmuls of subtile j-2)
        # goes into one FIFO of closures, drained inside the exp windows of
        # the score loop: PE never idles waiting for ACT, and never sees a
        # >3.4us gap (which would drop the HAM clock gate back to 1.2 GHz).
        filler = []
        popped = [0]
        appended = [0]

        def push(fn):
            filler.append(fn)
            appended[0] += 1

        def fill(n):
            while n > 0 and filler:
                filler.pop(0)()
                popped[0] += 1
                n -= 1

        def drain_to(mark):
            while popped[0] < mark and filler:
                filler.pop(0)()
                popped[0] += 1

        def proj_block(w_sb, rhs_of_c, done):
            ps_box = []

            def mk(c):
                def go():
                    if c == 0:
                        ps_box.append(proj_ps.tile([128, 512], F32,
                                                   name="pp", tag="pp"))
                    nc.tensor.matmul(ps_box[0][:], w_sb[:, c, :], rhs_of_c(c),
                                     start=(c == 0), stop=(c == 7))
                    if c == 7:
                        done(ps_box[0])
                return go

            for c in range(8):
                push(mk(c))

        def q_done(tb):
            def done(ps):
                nc.vector.tensor_copy(qT[:, ts(tb, 512)], ps[:])
            return done

        def k_done(sb):
            def done(ps):
                nc.vector.tensor_copy(kT[:, ts(sb, 512)], ps[:])
            return done

        def v_done(sb):
            def done(ps):
                vt = vt_pool.tile([128, 512], BF16)
                nc.vector.tensor_copy(vt[:], ps[:])
                nc.sync.dma_start_transpose(v_sb[:, 4 * sb:4 * sb + 4, :], vt[:])
            return done

        # supply schedule: k(sb) [q(tb)] v(sb); marks record the FIFO position
        # whose drain guarantees kT(sb) / qT(tb) writes are emitted (Tile
        # derives dependencies from emission order, so consumers must be
        # emitted after producers).
        k_mark = {}
        q_mark = {}
        for sb in range(8):
            proj_block(wk_sb, lambda c, s=sb: xt_sb[:, c, s, :], k_done(sb))
            k_mark[sb] = appended[0]
            if sb < 4:
                proj_block(wq_sb, lambda c, t=sb: xq_sb[:, c, t, :], q_done(sb))
                q_mark[sb] = appended[0]
            proj_block(wv_sb, lambda c, s=sb: xt_sb[:, c, s, :], v_done(sb))

        def attention_scores(j):
            nch = j // 2 + 1          # 512-wide key chunks covered
            ngr = (nch + 2) // 3      # 3-chunk exp groups per head
            p1 = p_pool.tile([128, nch, 512], BF16, tag="p1")
            p2 = p_pool.tile([128, nch, 512], BF16, tag="p2")
            sp1 = small.tile([128, 3], F32, tag="sp1")
            sp2 = small.tile([128, 3], F32, tag="sp2")
            for gi in range(ngr):
                used = min(3, nch - 3 * gi)
                for h, (p, sp) in ((0, (p1, sp1)), (1, (p2, sp2))):
                    sc = sc_ps.tile([128, 3, 512], F32, tag=f"sc{h}")
                    for qd in range(used):
                        ch = 3 * gi + qd
                        nc.tensor.matmul(
                            sc[:, qd, :],
                            qT[64 * h:64 * h + 64, ts(j, 128)],
                            kT[64 * h:64 * h + 64, ts(ch, 512)],
                            start=True, stop=True)
                    if 3 * gi + used == nch:  # strip's diagonal chunk
                        nc.vector.tensor_add(sc[:, used - 1, :],
                                             sc[:, used - 1, :],
                                             cm_sb[:, j % 2, :])
                    nc.scalar.activation(p[:, 3 * gi:3 * gi + used, :],
                                         sc[:, 0:used, :], Exp,
                                         accum_out=sp[:, gi:gi + 1])
                    # cover the (used*512+352)/1.2 ns exp window with ~200ns
                    # filler slots so PE stays busy while ACT runs
                    fill(max(2, (used * 512 + 352) // 220))
            if ngr == 1:
                sum1, sum2 = sp1[:, 0:1], sp2[:, 0:1]
            else:
                s1t = small.tile([128, 1], F32, tag="s1t")
                s2t = small.tile([128, 1], F32, tag="s2t")
                nc.vector.tensor_reduce(s1t[:], sp1[:, 0:ngr],
                                        axis=mybir.AxisListType.X, op=add)
                nc.vector.tensor_reduce(s2t[:], sp2[:, 0:ngr],
                                        axis=mybir.AxisListType.X, op=add)
                sum1, sum2 = s1t[:], s2t[:]
            r2 = small.tile([128, 1], F32, tag="r2")
            r1 = small.tile([128, 1], F32, tag="r1")
            gsc = small.tile([128, 1], F32, tag="gsc")
            nc.vector.reciprocal(r2[:], sum2)
            nc.vector.reciprocal(r1[:], sum1)
            # gsc = lam * sum1 / sum2
            nc.vector.scalar_tensor_tensor(gsc[:], sum1, float(lam), r2[:],
                                           op0=mult, op1=mult)
            # p_neg = p2 * gsc - p1   (one fused DVE pass over the strip)
            pn = p_pool.tile([128, nch, 512], BF16, tag="pn")
            nc.vector.scalar_tensor_tensor(pn[:], p2[:, 0:nch, :], gsc[:],
                                           p1[:, 0:nch, :], op0=mult, op1=sub)
            return pn, r1, nch

        def queue_pv(j, pt, r1, nch):
            pv_box = []

            def mk_mm(cc):
                def go():
                    if cc == 0:
                        pv_box.append(pv_ps.tile([128, 128], F32,
                                                 name="pv", tag="pv"))
                    nc.tensor.matmul(pv_box[0][:], pt[:, cc, :], v_sb[:, cc, :],
                                     start=(cc == 0), stop=(cc == 4 * nch - 1))
                return go

            def finish():
                osb = osb_pool.tile([128, 128], BF16)
                # out = pv * r1 * (-1)  (fused negate undoes the p_neg sign)
                nc.vector.tensor_scalar(osb[:], pv_box[0][:], r1[:], -1.0,
                                        op0=mult, op1=mult)
                nc.gpsimd.dma_start(out_e[j, :, :], osb[:])

            for cc in range(4 * nch):
                push(mk_mm(cc))
            push(finish)

        pending_t = []   # (j, pn, r1, nch) awaiting transpose emission
        pending_pv = []  # (j, pt, r1, nch) awaiting PV queueing

        def emit_transpose(ent):
            # lag-1 emission: pn is complete by now, so the xbar transpose
            # never head-of-line-blocks the sync HWDGE ring (which also
            # carries the v transposes PV depends on)
            j, pn, r1, nch = ent
            pt = pt_pool.tile([128, 4 * nch, 128], BF16, name="pt")
            nc.sync.dma_start_transpose(pt[:], pn[:])
            pending_pv.append((j, pt, r1, nch))

        def run_subtile(j):
            # kT/qT producer closures for this subtile must be emitted first
            drain_to(max(k_mark[j // 2], q_mark[j // 4]))
            if pending_t:
                emit_transpose(pending_t.pop(0))
            if len(pending_pv) >= 2:
                queue_pv(*pending_pv.pop(0))
            res = attention_scores(j)
            pending_t.append((j, *res))
            fill(4)

        for j in range(NSUB):
            run_subtile(j)
        while pending_t:
            emit_transpose(pending_t.pop(0))
        while pending_pv:
            queue_pv(*pending_pv.pop(0))
            fill(len(filler))
        fill(len(filler))

    nc.compile()
    return nc


def _lambda_init(depth):
    return 0.8 - 0.6 * math.exp(-0.3 * (depth + 1))


def kernel(x, Wq, Wk, Wv, lambda_q1, lambda_q2, lambda_k1, lambda_k2):
    global LAST_EXEC_NS
    x = np.asarray(x, dtype=np.float32)
    Wq = np.asarray(Wq, dtype=np.float32)
    Wk = np.asarray(Wk, dtype=np.float32)
    Wv = np.asarray(Wv, dtype=np.float32)
    lq1 = np.asarray(lambda_q1, dtype=np.float64)
    lq2 = np.asarray(lambda_q2, dtype=np.float64)
    lk1 = np.asarray(lambda_k1, dtype=np.float64)
    lk2 = np.asarray(lambda_k2, dtype=np.float64)

    lam = float(np.exp(np.dot(lq1, lk1)) - np.exp(np.dot(lq2, lk2))
                + _lambda_init(0))

    key = round(lam, 9)
    if key not in _NC_CACHE:
        _NC_CACHE[key] = _build(lam)
    nc = _NC_CACHE[key]

    def _wswz(w):  # [C, H2] -> [128p, 8c, 128]
        return np.ascontiguousarray(
            w.astype(NPBF16).reshape(8, 128, 128).transpose(1, 0, 2))

    wq_h = _wswz(Wq * 0.125)
    wk_h = _wswz(Wk)
    wv_h = _wswz(Wv)

    # x^T per batch in bf16, then swizzle to [128p, 8c, 8sb, 512]
    xT = [x[b].T.astype(NPBF16) for b in range(B)]
    xt_h = [np.ascontiguousarray(
        t.reshape(8, 128, 8, 512).transpose(1, 0, 2, 3)) for t in xT]

    i_idx = np.arange(128)[:, None]
    j_idx = np.arange(512)[None, :]
    in_maps = []
    for core in range(8):
        b, half = core // 2, core % 2
        qcols = np.concatenate(
            [np.arange(_t0(j, half), _t0(j, half) + 128) for j in range(NSUB)])
        xq = np.ascontiguousarray(
            xT[b][:, qcols].reshape(8, 128, 4, 512).transpose(1, 0, 2, 3))
        cm = np.empty((128, 2, 512), dtype=NPBF16)
        for m in range(2):
            r = 128 * m + 256 * half
            cm[:, m, :] = np.where(i_idx + r >= j_idx, 0.0,
                                   MASK_NEG).astype(NPBF16)
        in_maps.append({"xT": xt_h[b], "xq": xq, "wq": wq_h, "wk": wk_h,
                        "wv": wv_h, "cmask": cm})

    try:
        res = run_bass_kernel_spmd(nc, in_maps, list(range(8)))
    except Exception:
        if os.environ.get("BASS_TRACE"):
            # profiling path failed; rerun untraced
            os.environ["BASS_NEVER_TRACE"] = "1"
            res = run_bass_kernel_spmd(nc, in_maps, list(range(8)))
        else:
            raise
    LAST_EXEC_NS = res.exec_time_ns

    out = np.empty((B, T, H2), dtype=np.float32)
    for core in range(8):
        b, half = core // 2, core % 2
        o = np.asarray(res.results[core]["out"]).astype(np.float32)
        for j in range(NSUB):
            t0 = _t0(j, half)
            out[b, t0:t0 + 128, :] = o[j]
    return out
